# revision 15
# baseline (speedup 1.0000x reference)
"""Single-head attention kernel for Trainium2, SPMD over 8 NeuronCores.

Problem: x [4,4096,1024] f32 -> q/k/v = x@W+b (head 128) -> softmax(q k^T/sqrt(128)) @ v.
Sharding: core i handles batch i//2, query half i%2. Each core receives its
batch's x PRE-TRANSPOSED and PE-packed on the host (xTp [128, 8 sc, 8 ec, 512]
fp16, queries rotated to the front; key order is irrelevant to softmax sums);
all cores run one identical program.

Design (v4; NTFF-trace driven: v1 157.6us -> v2 147.4 -> v3 128.4):
- All layout work on the host: x ships transposed + packed so each
  512-column chunk is ONE DMA of 128x8KB contiguous descriptors (v3's
  1KB-descriptor pattern ran at ~139GB/s, this runs at full ~360GB/s);
  weights ship fp16 in stationary layout packed as [128, 3072] (2 DMAs);
  the three biases pack into one [128, 3] tensor (v3 spent 2.9us of queue
  time on three 4B-element DMAs).
- Projections: W chunk stationary, x^T chunk moving, fp32 PSUM, bias-add
  on VectorE doubles as the PSUM->SBUF f16 downcast.
- S^T = K[k,d] @ Q^T[d,q] per key-tile kt, emitted as soon as its kT chunk
  and q-span exist -- kt 0..3 go in 512-wide q-halves right after chunk 0
  so ScalarE (the second-busiest engine, ~74us of exp) starts ~16us in and
  never becomes the tail. exp spans [128,1024] otherwise, kt-ascending.
- P@V keeps P stationary / V+ones moving (softmax denominator lands free
  in PSUM col 128). Sweep 1 (4 accumulator banks, kt 0..27 kt-major,
  interleaved with the last chunk's s_exps, then kt 28..31); sweeps 2-3
  (6 banks) follow, the last acc-serial with inline normalize+DMA so the
  final output DMA trails the last matmul by ~1us.
- Whole compute path fp16 (1 cyc/row vs 4 for fp32) with fp32 PSUM;
  measured ~5e-4 end-to-end rel err.
"""

import sys

if "/opt/trn_rl_repo" not in sys.path:
    sys.path.insert(0, "/opt/trn_rl_repo")

import numpy as np

P = 128          # partitions
S = 4096         # sequence length
E = 1024         # n_embd
D = 128          # head size
SQ = 2048        # queries per core
SC = 512         # s-processing chunk (phase 1)
NSC = S // SC    # 8
NEC = E // P     # 8
NKT = S // P     # 32 key tiles
QH = 1024        # exp span (half of SQ)
SCALE = 1.0 / float(np.sqrt(D))

_CACHE = {}


def _build_nc():
    import concourse.mybir as mybir
    import concourse.tile as tile
    from concourse import bacc

    f32 = mybir.dt.float32
    f16 = mybir.dt.float16
    AF = mybir.ActivationFunctionType

    nc = bacc.Bacc(None, target_bir_lowering=False)
    xTp = nc.dram_tensor("xTp16", [P, NSC, NEC, SC], f16, kind="ExternalInput")
    wpk = nc.dram_tensor("wpack16", [P, 3 * E], f16, kind="ExternalInput")
    bqkv = nc.dram_tensor("bqkv", [P, 3], f32, kind="ExternalInput")
    ident = nc.dram_tensor("ident16", [P, P], f16, kind="ExternalInput")
    out = nc.dram_tensor("out", [SQ, D], f32, kind="ExternalOutput")

    with tile.TileContext(nc) as tc:
        with tc.tile_pool(name="const", bufs=1) as constp, \
             tc.tile_pool(name="big", bufs=1) as bigp, \
             tc.tile_pool(name="xfp", bufs=3) as xfp, \
             tc.tile_pool(name="vtmp", bufs=2) as vtmpp, \
             tc.tile_pool(name="pp", bufs=64) as pp, \
             tc.tile_pool(name="op", bufs=4) as op:

            # --- DMAs in the order compute needs them; first matmul only
            # needs Wq + x chunk 0 ---
            w_sb = constp.tile([P, 3 * E], f16, name="wpack")
            nc.sync.dma_start(out=w_sb[:, 0:2 * E], in_=wpk[:, 0:2 * E])  # Wq|Wk
            wq_sb, wk_sb, wv_sb = (w_sb[:, 0:E], w_sb[:, E:2 * E],
                                   w_sb[:, 2 * E:3 * E])
            # chunk 0 in two halves so sc0's projections pipeline with the DMA
            x_tiles = [xfp.tile([P, NEC, SC], f16, tag="xT", name="xT")]
            nc.sync.dma_start(out=x_tiles[0][:, 0:NEC // 2, :],
                              in_=xTp[:, 0, 0:NEC // 2, :])
            nc.sync.dma_start(out=x_tiles[0][:, NEC // 2:NEC, :],
                              in_=xTp[:, 0, NEC // 2:NEC, :])
            nc.sync.dma_start(out=w_sb[:, 2 * E:3 * E], in_=wpk[:, 2 * E:3 * E])  # Wv
            id16 = constp.tile([P, P], f16)
            nc.sync.dma_start(out=id16, in_=ident[:, :])
            b_sb = constp.tile([P, 3], f32, name="bqkv")
            nc.sync.dma_start(out=b_sb, in_=bqkv[:, :])
            bq_sb, bk_sb, bv_sb = b_sb[:, 0:1], b_sb[:, 1:2], b_sb[:, 2:3]

            # persistent activations (all fp16)
            kT_sb = bigp.tile([P, S], f16)        # K^T  [d, s]
            qT_sb = bigp.tile([P, SQ], f16)       # Q^T  [d, q]
            v_all = bigp.tile([P, NKT, D + 1], f16)  # [k_local, kt, 128 V | ones]
            nc.vector.memset(v_all[:, :, D:D + 1], 1.0)

            p_tiles = {}   # (qh, kt) -> [128 k, 1024 q] f16

            def p_tile(kt, qh):
                key = (qh, kt)
                if key not in p_tiles:
                    p_tiles[key] = pp.tile([P, QH], f16, tag="p", name="p")
                return p_tiles[key]

            def s_exp(kt, qh):
                sp = sp_ps.tile([P, QH], f32, tag="sp", name="sp")
                for h in range(QH // SC):
                    nc.tensor.matmul(sp[:, h * SC:(h + 1) * SC],
                                     kT_sb[:, kt * P:(kt + 1) * P],
                                     qT_sb[:, qh * QH + h * SC:
                                           qh * QH + (h + 1) * SC],
                                     start=True, stop=True)
                nc.scalar.activation(p_tile(kt, qh), sp, AF.Exp, scale=SCALE)

            def s_exp_half2(kta, ktb, qh, h):
                # one [128,1024] PSUM tile carries the 512-wide q-half S
                # blocks of TWO kt tiles; one exp per kt-half
                sp = sp_ps.tile([P, QH], f32, tag="sp", name="sp")
                for i, kt in enumerate((kta, ktb)):
                    nc.tensor.matmul(sp[:, i * SC:(i + 1) * SC],
                                     kT_sb[:, kt * P:(kt + 1) * P],
                                     qT_sb[:, qh * QH + h * SC:
                                           qh * QH + (h + 1) * SC],
                                     start=True, stop=True)
                for i, kt in enumerate((kta, ktb)):
                    nc.scalar.activation(
                        p_tile(kt, qh)[:, h * SC:(h + 1) * SC],
                        sp[:, i * SC:(i + 1) * SC], AF.Exp, scale=SCALE)

            def pv_mm(acc, qs, kt):
                # accs are packed 3-per-PSUM-bank over a memset-zeroed tile,
                # so every matmul accumulates (start=False) and the group
                # check (which expects a start=True leader) is skipped
                nc.tensor.matmul(
                    acc, p_tiles[(qs // 8, kt)][:, (qs % 8) * P:(qs % 8 + 1) * P],
                    v_all[:, kt, :], start=False, stop=(kt == NKT - 1),
                    skip_group_check=True)

            def out_chain(acc, qs):
                rec = op.tile([P, 1], f32, tag="rec", name="rec")
                nc.vector.reciprocal(rec, acc[:, D:D + 1])
                o_sb = op.tile([P, D], f32, tag="o", name="o")
                nc.vector.tensor_scalar_mul(o_sb, acc[:, 0:D], rec)
                nc.sync.dma_start(out=out[qs * P:(qs + 1) * P, :], in_=o_sb)

            # exp emission schedule per chunk: (kt, qh) pairs, kt-ascending.
            # kt 0..3 x qh0 go in 512 halves (h0 after sc0, h1 after sc1) via
            # s_exp_half2; qh0 of kt>=4 after max(sc(kt),1); qh1 after
            # max(sc(kt),3).
            exp_sched = {sc: [] for sc in range(NSC)}
            for kt in range(4, 16):
                exp_sched[max(kt // 4, 1)].append((kt, 0))
                exp_sched[3].append((kt, 1))
            for kt in range(0, 4):
                exp_sched[3].append((kt, 1))
            for kt in range(16, NKT):
                exp_sched[kt // 4].append((kt, 0))
                exp_sched[kt // 4].append((kt, 1))
            for sc in exp_sched:
                exp_sched[sc].sort()

            # ---------------- phase 1: projections + interleaved S/exp ----------------
            sp_cm = tc.tile_pool(name="sp_ps", bufs=2, space="PSUM")
            proj_cm = tc.tile_pool(name="proj_ps", bufs=1, space="PSUM")
            vt_cm = tc.tile_pool(name="vt_ps", bufs=1, space="PSUM")
            sp_ps, proj_ps, vt_ps = (sp_cm.__enter__(), proj_cm.__enter__(),
                                     vt_cm.__enter__())
            for sc in range(NSC):
                if sc > 0:
                    xt = xfp.tile([P, NEC, SC], f16, tag="xT", name="xT")
                    nc.sync.dma_start(out=xt, in_=xTp[:, sc, :, :])
                    x_tiles.append(xt)
                xt = x_tiles[sc]

                def proj(w_ap, tag):
                    ps = proj_ps.tile([P, SC], f32, tag=tag, name=tag)
                    for ec in range(NEC):
                        nc.tensor.matmul(ps, w_ap[:, ec * P:(ec + 1) * P],
                                         xt[:, ec, :],
                                         start=(ec == 0), stop=(ec == NEC - 1))
                    return ps

                # Q,K first so the chunk's S/exp work can start before V
                if sc < 4:
                    pq = proj(wq_sb, "pq")
                    nc.vector.tensor_scalar_add(
                        qT_sb[:, sc * SC:(sc + 1) * SC], pq, bq_sb)
                pk = proj(wk_sb, "pk")
                nc.vector.tensor_scalar_add(
                    kT_sb[:, sc * SC:(sc + 1) * SC], pk, bk_sb)
                if sc < 2:
                    # earliest exps: kt 0..3 x q-half (sc0: h0, sc1: h1)
                    s_exp_half2(0, 1, 0, sc)
                    s_exp_half2(2, 3, 0, sc)
                if sc < NSC - 1:
                    for kt, qh in exp_sched[sc]:
                        s_exp(kt, qh)
                # V: bias add (f32 psum -> f16), PE transpose, pack
                ps = proj(wv_sb, "pv")
                vtmp = vtmpp.tile([P, SC], f16, tag="vtmp", name="vtmp")
                nc.vector.tensor_scalar_add(vtmp, ps, bv_sb)
                vt = vt_ps.tile([P, SC], f16, tag="vt", name="vt")
                for i in range(4):
                    nc.tensor.transpose(vt[:, i * P:(i + 1) * P],
                                        vtmp[:, i * P:(i + 1) * P],
                                        id16)
                nc.vector.tensor_copy(
                    v_all[:, sc * 4:(sc + 1) * 4, 0:D],
                    vt[:, :].rearrange("p (b c) -> p b c", c=P))
            vt_cm.__exit__(None, None, None)
            proj_cm.__exit__(None, None, None)

            # ---------------- phase 2: P@V ----------------
            # 12 accumulators packed 3-per-bank into the 4 banks freed by
            # proj/vt run kt 0..27 BEFORE the exp tail finishes (interleaved
            # with the last chunk's s_exp pairs); only kt 28..31 and the
            # last 4 accumulators trail the final exp.
            tail = exp_sched[NSC - 1]
            with tc.tile_pool(name="acc1_ps", bufs=1, space="PSUM") as acc1:
                at = [acc1.tile([P, 3, D + 1], f32, tag=f"acc{t}", name="acc")
                      for t in range(4)]
                for t in range(4):
                    nc.vector.memset(at[t], 0.0)
                accs = {qs: at[qs // 3][:, qs % 3, :] for qs in range(12)}
                for g in range(4):
                    s_exp(*tail[2 * g])
                    s_exp(*tail[2 * g + 1])
                    for kt in range(g * 7, (g + 1) * 7):
                        for qs in range(12):
                            pv_mm(accs[qs], qs, kt)
                for kt in range(28, NKT):
                    for qs in range(12):
                        pv_mm(accs[qs], qs, kt)
                for qs in range(12):
                    out_chain(accs[qs], qs)
            sp_cm.__exit__(None, None, None)

            with tc.tile_pool(name="acc2_ps", bufs=1, space="PSUM") as acc2:
                # last 4 accumulators: acc-serial with inline
                # normalize+store so the final output DMA trails the
                # last matmul by ~1us only
                a2 = acc2.tile([P, 3, D + 1], f32, tag="acc4", name="acc")
                a3 = acc2.tile([P, 1, D + 1], f32, tag="acc5", name="acc")
                nc.vector.memset(a2, 0.0)
                nc.vector.memset(a3, 0.0)
                for qs in range(12, 16):
                    acc = a2[:, qs - 12, :] if qs < 15 else a3[:, 0, :]
                    for kt in range(NKT):
                        pv_mm(acc, qs, kt)
                    out_chain(acc, qs)
    nc.finalize()
    return nc


def _get_nc():
    if "nc" not in _CACHE:
        _CACHE["nc"] = _build_nc()
    return _CACHE["nc"]


def _pack_w(w):
    # [1024, 128] -> stationary layout [128, 8*128]: chunk ec on free axis
    w16 = np.asarray(w, np.float32).astype(np.float16)
    return w16.reshape(NEC, P, D).transpose(1, 0, 2).reshape(P, E)


def _in_maps(x, Wq, bq, Wk, bk, Wv, bv):
    x = np.asarray(x, dtype=np.float32).astype(np.float16)
    shared = {
        "wpack16": np.ascontiguousarray(
            np.concatenate([_pack_w(Wq), _pack_w(Wk), _pack_w(Wv)], axis=1)),
        "bqkv": np.ascontiguousarray(np.stack(
            [np.asarray(b, np.float32) for b in (bq, bk, bv)], axis=1)),
        "ident16": np.eye(P, dtype=np.float16),
    }
    maps = []
    for core in range(8):
        b, h = core // 2, core % 2
        xb = x[b] if h == 0 else np.concatenate([x[b, SQ:], x[b, :SQ]], axis=0)
        # x^T [1024, 4096] -> [128(p), 8(sc), 8(ec), 512]: one 8KB-contiguous
        # read per partition per chunk DMA
        xtp = xb.T.reshape(NEC, P, NSC, SC).transpose(1, 2, 0, 3)
        maps.append({"xTp16": np.ascontiguousarray(xtp), **shared})
    return maps


def _assemble(results):
    out = np.empty((4, S, D), dtype=np.float32)
    for core in range(8):
        b, h = core // 2, core % 2
        out[b, h * SQ:(h + 1) * SQ] = results[core]["out"]
    return out


def kernel(x, Wq, bq, Wk, bk, Wv, bv):
    from concourse.bass_utils import run_bass_kernel_spmd

    nc = _get_nc()
    res = run_bass_kernel_spmd(nc, _in_maps(x, Wq, bq, Wk, bk, Wv, bv),
                               core_ids=list(range(8)))
    return _assemble(res.results)


# revision 21
# speedup vs baseline: 1.0997x; 1.0997x over previous
"""Single-head attention kernel for Trainium2, SPMD over 8 NeuronCores.

Problem: x [4,4096,1024] f32 -> q/k/v = x@W+b (head 128) -> softmax(q k^T/sqrt(128)) @ v.
Sharding: core i handles batch i//2, query half i%2. Each core receives its
batch's x PRE-TRANSPOSED and PE-packed on the host (xTp [128, 8 sc, 8 ec, 512]
fp16, queries rotated to the front; key order is irrelevant to softmax sums);
all cores run one identical program.

Design (v4; NTFF-trace driven: v1 157.6us -> v2 147.4 -> v3 128.4):
- All layout work on the host: x ships transposed + packed so each
  512-column chunk is ONE DMA of 128x8KB contiguous descriptors (v3's
  1KB-descriptor pattern ran at ~139GB/s, this runs at full ~360GB/s);
  weights ship fp16 in stationary layout packed as [128, 3072] (2 DMAs);
  the three biases pack into one [128, 3] tensor (v3 spent 2.9us of queue
  time on three 4B-element DMAs).
- Projections: W chunk stationary, x^T chunk moving, fp32 PSUM, bias-add
  on VectorE doubles as the PSUM->SBUF f16 downcast.
- S^T = K[k,d] @ Q^T[d,q] per key-tile kt, emitted as soon as its kT chunk
  and q-span exist -- kt 0..3 go in 512-wide q-halves right after chunk 0
  so ScalarE (the second-busiest engine, ~74us of exp) starts ~16us in and
  never becomes the tail. exp spans [128,1024] otherwise, kt-ascending.
- P@V keeps P stationary / V+ones moving (softmax denominator lands free
  in PSUM col 128). Sweep 1 (4 accumulator banks, kt 0..27 kt-major,
  interleaved with the last chunk's s_exps, then kt 28..31); sweeps 2-3
  (6 banks) follow, the last acc-serial with inline normalize+DMA so the
  final output DMA trails the last matmul by ~1us.
- Whole compute path fp16 (1 cyc/row vs 4 for fp32) with fp32 PSUM;
  measured ~5e-4 end-to-end rel err.
"""

import sys

if "/opt/trn_rl_repo" not in sys.path:
    sys.path.insert(0, "/opt/trn_rl_repo")

import numpy as np

P = 128          # partitions
S = 4096         # sequence length
E = 1024         # n_embd
D = 128          # head size
SQ = 2048        # queries per core
SC = 512         # s-processing chunk (phase 1)
NSC = S // SC    # 8
NEC = E // P     # 8
NKT = S // P     # 32 key tiles
QH = 1024        # exp span (half of SQ)
SCALE = 1.0 / float(np.sqrt(D))

_CACHE = {}


def _build_nc():
    import concourse.mybir as mybir
    import concourse.tile as tile
    from concourse import bacc

    f32 = mybir.dt.float32
    f16 = mybir.dt.float16
    AF = mybir.ActivationFunctionType

    nc = bacc.Bacc(None, target_bir_lowering=False)
    xTp = nc.dram_tensor("xTp16", [P, NSC, NEC, SC], f16, kind="ExternalInput")
    wpk = nc.dram_tensor("wpack16", [P, 3 * E], f16, kind="ExternalInput")
    bqkv = nc.dram_tensor("bqkv", [P, 3], f32, kind="ExternalInput")
    ident = nc.dram_tensor("ident16", [P, P], f16, kind="ExternalInput")
    # p-major output layout: out[p, qs, d] holds query qs*128+p; the host
    # un-packs. This lets the 16 output blocks batch into 4 DMAs whose
    # descriptor rows are >=512B-contiguous per partition.
    out = nc.dram_tensor("out", [P, SQ // P, D], f32, kind="ExternalOutput")

    with tile.TileContext(nc) as tc:
        with tc.tile_pool(name="const", bufs=1) as constp, \
             tc.tile_pool(name="big", bufs=1) as bigp, \
             tc.tile_pool(name="xfp", bufs=3) as xfp, \
             tc.tile_pool(name="vtmp", bufs=2) as vtmpp, \
             tc.tile_pool(name="pp", bufs=64) as pp, \
             tc.tile_pool(name="ogp", bufs=1) as ogp, \
             tc.tile_pool(name="op", bufs=4) as op:

            # --- DMAs in the order compute needs them; first matmul only
            # needs Wq + x chunk 0 ---
            w_sb = constp.tile([P, 3 * E], f16, name="wpack")
            nc.sync.dma_start(out=w_sb[:, 0:E], in_=wpk[:, 0:E])          # Wq
            wq_sb, wk_sb, wv_sb = (w_sb[:, 0:E], w_sb[:, E:2 * E],
                                   w_sb[:, 2 * E:3 * E])
            x_tiles = [xfp.tile([P, NEC, SC], f16, tag="xT", name="xT")]
            nc.sync.dma_start(out=x_tiles[0], in_=xTp[:, 0, :, :])
            nc.sync.dma_start(out=w_sb[:, E:2 * E], in_=wpk[:, E:2 * E])  # Wk
            nc.sync.dma_start(out=w_sb[:, 2 * E:3 * E], in_=wpk[:, 2 * E:3 * E])  # Wv
            id16 = constp.tile([P, P], f16)
            nc.sync.dma_start(out=id16, in_=ident[:, :])
            b_sb = constp.tile([P, 3], f32, name="bqkv")
            nc.sync.dma_start(out=b_sb, in_=bqkv[:, :])
            bq_sb, bk_sb, bv_sb = b_sb[:, 0:1], b_sb[:, 1:2], b_sb[:, 2:3]

            # persistent activations (all fp16)
            kT_sb = bigp.tile([P, S], f16)        # K^T  [d, s]
            qT_sb = bigp.tile([P, SQ], f16)       # Q^T  [d, q]
            v_all = bigp.tile([P, NKT, D + 1], f16)  # [k_local, kt, 128 V | ones]
            nc.vector.memset(v_all[:, :, D:D + 1], 1.0)

            p_tiles = {}   # (qh, kt) -> [128 k, 1024 q] f16

            def p_tile(kt, qh):
                key = (qh, kt)
                if key not in p_tiles:
                    p_tiles[key] = pp.tile([P, QH], f16, tag="p", name="p")
                return p_tiles[key]

            def s_exp(kt, qh):
                sp = sp_ps.tile([P, QH], f32, tag="sp", name="sp")
                for h in range(QH // SC):
                    nc.tensor.matmul(sp[:, h * SC:(h + 1) * SC],
                                     kT_sb[:, kt * P:(kt + 1) * P],
                                     qT_sb[:, qh * QH + h * SC:
                                           qh * QH + (h + 1) * SC],
                                     start=True, stop=True)
                nc.scalar.activation(p_tile(kt, qh), sp, AF.Exp, scale=SCALE)

            def s_exp_half2(kta, ktb, qh, h):
                # one [128,1024] PSUM tile carries the 512-wide q-half S
                # blocks of TWO kt tiles; one exp per kt-half
                sp = sp_ps.tile([P, QH], f32, tag="sp", name="sp")
                for i, kt in enumerate((kta, ktb)):
                    nc.tensor.matmul(sp[:, i * SC:(i + 1) * SC],
                                     kT_sb[:, kt * P:(kt + 1) * P],
                                     qT_sb[:, qh * QH + h * SC:
                                           qh * QH + (h + 1) * SC],
                                     start=True, stop=True)
                for i, kt in enumerate((kta, ktb)):
                    nc.scalar.activation(
                        p_tile(kt, qh)[:, h * SC:(h + 1) * SC],
                        sp[:, i * SC:(i + 1) * SC], AF.Exp, scale=SCALE)

            def pv_mm(acc, qs, kt, packed=True):
                # packed accs share a memset-zeroed PSUM bank 3-wide, so
                # every matmul accumulates (start=False); bank-exclusive
                # accs use the normal start=True-clears-bank protocol
                nc.tensor.matmul(
                    acc, p_tiles[(qs // 8, kt)][:, (qs % 8) * P:(qs % 8 + 1) * P],
                    v_all[:, kt, :], start=(False if packed else kt == 0),
                    stop=(kt == NKT - 1), skip_group_check=packed)

            def out_chain(acc, qs, og):
                # normalize: reciprocal on Vector; the multiply alternates
                # Vector/Scalar so neither engine serializes the 16 chains
                rec = op.tile([P, 1], f32, tag="rec", name="rec")
                nc.vector.reciprocal(rec, acc[:, D:D + 1])
                dst = og[:, qs % 4, :]
                if qs % 2 == 0:
                    nc.vector.tensor_scalar_mul(dst, acc[:, 0:D], rec)
                else:
                    nc.scalar.activation(dst, acc[:, 0:D], AF.Copy, scale=rec)

            # exp emission schedule per chunk: (kt, qh) pairs, kt-ascending.
            # kt 0..3 x qh0 go in 512 halves (h0 after sc0, h1 after sc1) via
            # s_exp_half2; qh0 of kt>=4 after max(sc(kt),1); qh1 after
            # max(sc(kt),3).
            exp_sched = {sc: [] for sc in range(NSC)}
            for kt in range(4, 16):
                exp_sched[max(kt // 4, 1)].append((kt, 0))
                exp_sched[3].append((kt, 1))
            for kt in range(0, 4):
                exp_sched[3].append((kt, 1))
            for kt in range(16, NKT):
                exp_sched[kt // 4].append((kt, 0))
                exp_sched[kt // 4].append((kt, 1))
            for sc in exp_sched:
                exp_sched[sc].sort()

            # ---------------- phase 1: projections + interleaved S/exp ----------------
            sp_cm = tc.tile_pool(name="sp_ps", bufs=2, space="PSUM")
            proj_cm = tc.tile_pool(name="proj_ps", bufs=1, space="PSUM")
            vt_cm = tc.tile_pool(name="vt_ps", bufs=1, space="PSUM")
            sp_ps, proj_ps, vt_ps = (sp_cm.__enter__(), proj_cm.__enter__(),
                                     vt_cm.__enter__())
            for sc in range(NSC):
                if sc > 0:
                    xt = xfp.tile([P, NEC, SC], f16, tag="xT", name="xT")
                    nc.sync.dma_start(out=xt, in_=xTp[:, sc, :, :])
                    x_tiles.append(xt)
                xt = x_tiles[sc]

                def proj(w_ap, tag):
                    ps = proj_ps.tile([P, SC], f32, tag=tag, name=tag)
                    for ec in range(NEC):
                        nc.tensor.matmul(ps, w_ap[:, ec * P:(ec + 1) * P],
                                         xt[:, ec, :],
                                         start=(ec == 0), stop=(ec == NEC - 1))
                    return ps

                # Q,K first so the chunk's S/exp work can start before V
                if sc < 4:
                    pq = proj(wq_sb, "pq")
                    nc.vector.tensor_scalar_add(
                        qT_sb[:, sc * SC:(sc + 1) * SC], pq, bq_sb)
                pk = proj(wk_sb, "pk")
                nc.vector.tensor_scalar_add(
                    kT_sb[:, sc * SC:(sc + 1) * SC], pk, bk_sb)
                if sc < 2:
                    # earliest exps: kt 0..3 x q-half (sc0: h0, sc1: h1)
                    s_exp_half2(0, 1, 0, sc)
                    s_exp_half2(2, 3, 0, sc)
                if sc < NSC - 1:
                    for kt, qh in exp_sched[sc]:
                        s_exp(kt, qh)
                # V: bias add (f32 psum -> f16), PE transpose, pack
                ps = proj(wv_sb, "pv")
                vtmp = vtmpp.tile([P, SC], f16, tag="vtmp", name="vtmp")
                nc.vector.tensor_scalar_add(vtmp, ps, bv_sb)
                vt = vt_ps.tile([P, SC], f16, tag="vt", name="vt")
                for i in range(4):
                    nc.tensor.transpose(vt[:, i * P:(i + 1) * P],
                                        vtmp[:, i * P:(i + 1) * P],
                                        id16)
                nc.vector.tensor_copy(
                    v_all[:, sc * 4:(sc + 1) * 4, 0:D],
                    vt[:, :].rearrange("p (b c) -> p b c", c=P))
            vt_cm.__exit__(None, None, None)
            proj_cm.__exit__(None, None, None)

            # ---------------- phase 2: P@V ----------------
            # 12 accumulators packed 3-per-bank into the 4 banks freed by
            # proj/vt run kt 0..27 BEFORE the exp tail finishes (interleaved
            # with the last chunk's s_exp pairs); only kt 28..31 and the
            # last 4 accumulators trail the final exp.
            tail = exp_sched[NSC - 1]
            with tc.tile_pool(name="acc1_ps", bufs=1, space="PSUM") as acc1:
                at = [acc1.tile([P, 3, D + 1], f32, tag=f"acc{t}", name="acc")
                      for t in range(4)]
                for t in range(4):
                    nc.vector.memset(at[t], 0.0)
                accs = {qs: at[qs // 3][:, qs % 3, :] for qs in range(12)}
                for g in range(4):
                    s_exp(*tail[2 * g])
                    s_exp(*tail[2 * g + 1])
                    for kt in range(g * 7, (g + 1) * 7):
                        for qs in range(12):
                            pv_mm(accs[qs], qs, kt)
                for kt in range(28, NKT):
                    for qs in range(12):
                        pv_mm(accs[qs], qs, kt)
                for g in range(3):
                    og = ogp.tile([P, 4, D], f32, tag=f"og{g}", name="og")
                    for qs in range(g * 4, (g + 1) * 4):
                        out_chain(accs[qs], qs, og)
                    nc.sync.dma_start(out=out[:, g * 4:(g + 1) * 4, :], in_=og)
            sp_cm.__exit__(None, None, None)

            with tc.tile_pool(name="acc2_ps", bufs=1, space="PSUM") as acc2:
                # last 4 accumulators: bank-exclusive (classic start=True),
                # acc-serial with inline normalize so the final output DMA
                # trails the last matmul by ~1us only
                og = ogp.tile([P, 4, D], f32, tag="og3", name="og")
                for qs in range(12, 16):
                    acc = acc2.tile([P, D + 1], f32, tag=f"acc{qs}", name="acc")
                    for kt in range(NKT):
                        pv_mm(acc, qs, kt, packed=False)
                    out_chain(acc, qs, og)
                nc.sync.dma_start(out=out[:, 12:16, :], in_=og)
    nc.finalize()
    return nc


def _get_nc():
    if "nc" not in _CACHE:
        _CACHE["nc"] = _build_nc()
    return _CACHE["nc"]


def _pack_w(w):
    # [1024, 128] -> stationary layout [128, 8*128]: chunk ec on free axis
    w16 = np.asarray(w, np.float32).astype(np.float16)
    return w16.reshape(NEC, P, D).transpose(1, 0, 2).reshape(P, E)


def _in_maps(x, Wq, bq, Wk, bk, Wv, bv):
    x = np.asarray(x, dtype=np.float32).astype(np.float16)
    shared = {
        "wpack16": np.ascontiguousarray(
            np.concatenate([_pack_w(Wq), _pack_w(Wk), _pack_w(Wv)], axis=1)),
        "bqkv": np.ascontiguousarray(np.stack(
            [np.asarray(b, np.float32) for b in (bq, bk, bv)], axis=1)),
        "ident16": np.eye(P, dtype=np.float16),
    }
    maps = []
    for core in range(8):
        b, h = core // 2, core % 2
        xb = x[b] if h == 0 else np.concatenate([x[b, SQ:], x[b, :SQ]], axis=0)
        # x^T [1024, 4096] -> [128(p), 8(sc), 8(ec), 512]: one 8KB-contiguous
        # read per partition per chunk DMA
        xtp = xb.T.reshape(NEC, P, NSC, SC).transpose(1, 2, 0, 3)
        maps.append({"xTp16": np.ascontiguousarray(xtp), **shared})
    return maps


def _assemble(results):
    out = np.empty((4, S, D), dtype=np.float32)
    for core in range(8):
        b, h = core // 2, core % 2
        # device output is p-major [128, 16, 128]: query qs*128+p at [p, qs]
        o = results[core]["out"].transpose(1, 0, 2).reshape(SQ, D)
        out[b, h * SQ:(h + 1) * SQ] = o
    return out


def kernel(x, Wq, bq, Wk, bk, Wv, bv):
    from concourse.bass_utils import run_bass_kernel_spmd

    nc = _get_nc()
    res = run_bass_kernel_spmd(nc, _in_maps(x, Wq, bq, Wk, bk, Wv, bv),
                               core_ids=list(range(8)))
    return _assemble(res.results)
